# revision 20
# baseline (speedup 1.0000x reference)
"""Trainium2 Bass kernel for nn_Bert_44452911514066 (DeBERTa-style disentangled
attention BERT layer), data-parallel over batch across 8 NeuronCores.

kernel(**inputs) takes the FULL inputs (as produced by reference.setup_inputs)
and returns the FULL [S, B, H] output.

v2 structure (vs v1):
  - attention runs per HEAD-PAIR: even head in partitions 0-63, odd in 64-127;
    all K=64 matmuls (expansions, CC) issue back-to-back for the two halves so
    the PE runs them concurrently in disjoint row-groups.
  - the content-content scores ride the cq expansion matmul as 512 extra
    columns (same stationary q-tile weights), landing [q, k]; a DMA with
    accum_op=add folds them onto the sheared cq tile, so the whole score
    tile goes through the same f16 PE transposes into [k, q].
  - f16 transposes (into an f16 PSUM bank) replace fp32 ones; the ck term is
    added by a shear-DMA with accum_op=add; no PE identity-matmuls.
  - emission is software-pipelined (PV of pair i-1 fills pair i's eviction
    waits), PSUM plan: xq_e(3) + xq_o(3) + tr(1) + pv(1) = 8 banks.
  - phase 1 overlaps LN1 (S/V split) with the position-table matmuls so the
    PE enters phase 2 warm.
"""
import sys
sys.path.insert(0, "/opt/trn_rl_repo")
import math
import functools
import contextlib
import numpy as np

import concourse.bass as bass
import concourse.tile as tile
from concourse import mybir
from concourse.masks import make_identity

H, NH, HD, S, B = 768, 12, 64, 512, 16
NCORES = 8
BL = B // NCORES          # batches per core
T = BL * S                # tokens per core
SCALE = 1.0 / math.sqrt(3 * HD)
EPS = 1e-7
NB = 63                   # relative buckets
WIN = 640                 # expansion window per 128-row tile
CSHIFT = 12.0             # exp shift
F16 = mybir.dt.float16
F32 = mybir.dt.float32
AF = mybir.ActivationFunctionType
OP = mybir.AluOpType

# ---------------------------------------------------------------------------
# walrus workaround: this container's walrus accepts at most ONE sync wait per
# instruction; split extra waits onto single-wait NoOps.
# ---------------------------------------------------------------------------
from concourse.vector_clock import ScopedClock

_orig_add_instruction = tile.TileContext._add_instruction


def _patched_add_instruction(self, inst):
    si = inst.sync_info
    if si is not None and si.on_wait is not None and len(si.on_wait) > 1:
        waits = list(si.on_wait)
        for i, w in enumerate(waits[:-1]):
            nop = mybir.InstNoOp(name=f"{inst.name}-wsplit{i}", ins=[], outs=[])
            nop.engine = inst.engine
            nop.sync_info = mybir.SyncInfo(on_wait=[w], on_update=[])
            _orig_add_instruction(self, nop)
        inst.sync_info = mybir.SyncInfo(
            on_wait=[waits[-1]], on_update=list(si.on_update or []))
    _orig_add_instruction(self, inst)


def _patched_drain_and_barrier(self, tick_clock, wait_clock):
    nc = self.nc
    probe = nc.sync.nop(nofuse=True)
    wait_clock.add_sem_waits(probe.ins, ScopedClock({None: tick_clock.global_clock}))
    si = probe.ins.sync_info
    waits = list(si.on_wait) if si is not None and si.on_wait else []
    if len(waits) > 1:
        probe.ins.sync_info = mybir.SyncInfo(on_wait=waits[:1], on_update=[])
        for w in waits[1:]:
            n2 = nc.sync.nop(nofuse=True)
            n2.ins.sync_info = mybir.SyncInfo(on_wait=[w], on_update=[])
    nc.sync.drain()
    nc.all_engine_barrier()
    assert self.sems is not None
    popped = nc._tile_sem_poison_stack.pop()
    assert popped is self._sem_poison
    nc.clear_and_free_semaphores(list(self.sems.allocated().values()))
    nc.all_engine_barrier()


tile.TileContext._add_instruction = _patched_add_instruction
tile.TileContext._drain_and_barrier = _patched_drain_and_barrier


# ---------------------------------------------------------------------------
# device kernel build
# ---------------------------------------------------------------------------
@functools.lru_cache(maxsize=4)
def build_module(with_bias: bool, debug: bool = False):
    nc = bass.Bass()
    if debug:
        dbg_qk_d = nc.dram_tensor("dbg_qk", [128, 12 * T], F16, kind="ExternalOutput")
        dbg_va_d = nc.dram_tensor("dbg_va", [128, 8 * NH * 65], F16, kind="ExternalOutput")
        dbg_g_d = nc.dram_tensor("dbg_g", [128, 8 * H], F16, kind="ExternalOutput")
        dbg_ctx_d = nc.dram_tensor("dbg_ctx", [128, 8 * H], F16, kind="ExternalOutput")
        dbg_et_d = nc.dram_tensor("dbg_et", [128, 4 * 2 * 512], F16, kind="ExternalOutput")
        dbg_cq_d = nc.dram_tensor("dbg_cq", [128, 4 * 2 * 512], F16, kind="ExternalOutput")

    hid_d = nc.dram_tensor("hid", [T, H], F32, kind="ExternalInput")
    wqkT_d = nc.dram_tensor("wqkT", [H, 2 * H], F16, kind="ExternalInput")
    wvgT_d = nc.dram_tensor("wvgT", [H, 2 * H], F16, kind="ExternalInput")
    woutT_d = nc.dram_tensor("woutT", [H, H], F16, kind="ExternalInput")
    relT_d = nc.dram_tensor("relT", [H, NB], F16, kind="ExternalInput")
    Ecq_d = nc.dram_tensor("Ecq", [NB, 1024], F16, kind="ExternalInput")
    Eck_d = nc.dram_tensor("Eck", [NB, 1024], F16, kind="ExternalInput")
    vmask_d = nc.dram_tensor("vmask", [T, 1], F32, kind="ExternalInput")
    if with_bias:
        bqkc_d = nc.dram_tensor("bqkc", [128, 12], F32, kind="ExternalInput")
        bqkr_d = nc.dram_tensor("bqkr", [1, 2 * H], F32, kind="ExternalInput")
        bvgr_d = nc.dram_tensor("bvgr", [1, 2 * H], F32, kind="ExternalInput")
        boutr_d = nc.dram_tensor("boutr", [1, H], F32, kind="ExternalInput")
    out_d = nc.dram_tensor("out", [T, H], F32, kind="ExternalOutput")
    h_dram = nc.dram_tensor("h_scratch", [T, H], F16)
    ln2_dram = nc.dram_tensor("ln2_scratch", [T, H], F16)

    with tile.TileContext(nc) as tc, contextlib.ExitStack() as ctx:
        persist = ctx.enter_context(tc.tile_pool(name="persist", bufs=1))
        stats = ctx.enter_context(tc.tile_pool(name="stats", bufs=4))

        # --- constants ---
        ident16 = persist.tile([128, 128], F16, tag="id16")
        make_identity(nc, ident16)
        eps_t = persist.tile([128, 1], F32, tag="eps")
        nc.vector.memset(eps_t, EPS)
        negc_t = persist.tile([128, 1], F32, tag="negc")
        nc.vector.memset(negc_t, -CSHIFT)

        woutT = persist.tile([128, 6, H], F16, tag="woutT")
        relT = persist.tile([128, 6, NB], F16, tag="relT")
        vmask16 = persist.tile([128, 8], F32, tag="vm")
        nc.sync.dma_start(
            out=vmask16[:],
            in_=vmask_d[:].rearrange("(t p) one -> p (t one)", p=128))
        if with_bias:
            bqkc = persist.tile([128, 12], F32, tag="bqkc")
            nc.sync.dma_start(out=bqkc[:], in_=bqkc_d[:])
            bqkr = persist.tile([64, 2 * H], F32, tag="bqkr")
            nc.sync.dma_start(
                out=bqkr[:],
                in_=bass.AP(tensor=bqkr_d, offset=0, ap=[[0, 64], [1, 2 * H]]))
            bvgr = persist.tile([128, 2 * H], F32, tag="bvgr")
            nc.sync.dma_start(
                out=bvgr[:],
                in_=bass.AP(tensor=bvgr_d, offset=0, ap=[[0, 128], [1, 2 * H]]))
            boutr = persist.tile([128, H], F32, tag="boutr")
            nc.sync.dma_start(
                out=boutr[:],
                in_=bass.AP(tensor=boutr_d, offset=0, ap=[[0, 128], [1, H]]))

        def layernorm(out16, xin, stt_engine):
            st = stats.tile([128, 3, 6], F32, tag="bnst")
            for sg in range(3):
                nc.vector.bn_stats(out=st[:, sg, :], in_=xin[:, 256 * sg:256 * sg + 256])
            mv = stats.tile([128, 2], F32, tag="bnmv")
            nc.vector.bn_aggr(out=mv[:], in_=st[:])
            rstd = stats.tile([128, 1], F32, tag="rstd")
            nc.scalar.activation(out=rstd[:], in_=mv[:, 1:2], func=AF.Sqrt,
                                 bias=eps_t[:], scale=1.0)
            nc.vector.reciprocal(out=rstd[:], in_=rstd[:])
            if stt_engine == "vector":
                nc.vector.scalar_tensor_tensor(
                    out=out16, in0=xin, scalar=mv[:, 0:1],
                    in1=rstd[:].to_broadcast((128, H)),
                    op0=OP.subtract, op1=OP.mult)
            else:
                # (x - m)*rstd == Identity(rstd*x + (-m*rstd)) on ScalarE
                nmr = stats.tile([128, 1], F32, tag="nmr")
                nc.vector.scalar_tensor_tensor(
                    out=nmr[:], in0=mv[:, 0:1], scalar=-1.0,
                    in1=rstd[:], op0=OP.mult, op1=OP.mult)
                nc.scalar.activation(out=out16, in_=xin, func=AF.Identity,
                                     bias=nmr[:], scale=rstd[:])

        posp = persist.tile([64, 2 * H], F16, tag="posp")
        Mh = persist.tile([128, 6, 1024], F16, tag="Mh")
        Mq = persist.tile([128, 6, 1024], F16, tag="Mq")
        qk16 = persist.tile([128, 12, T], F16, tag="qk16")
        g16 = persist.tile([128, 8, H], F16, tag="g16")
        va16 = persist.tile([128, 8, NH * 65], F16, tag="va16")
        ctx16 = persist.tile([128, 8, H], F16, tag="ctx16")

        # ================= phases 1-2 (weights pool scoped) ================
        with tc.tile_pool(name="ph12w", bufs=1) as ph12w:
            wqkT = ph12w.tile([128, 6, 2 * H], F16, tag="wqkT")
            wvgT = ph12w.tile([128, 6, 2 * H], F16, tag="wvgT")
            for c in range(6):
                nc.sync.dma_start(out=wqkT[:, c, :], in_=wqkT_d[128 * c:128 * c + 128, :])
                nc.sync.dma_start(out=wvgT[:, c, :], in_=wvgT_d[128 * c:128 * c + 128, :])
                nc.sync.dma_start(out=woutT[:, c, :], in_=woutT_d[128 * c:128 * c + 128, :])
                nc.sync.dma_start(out=relT[:, c, :], in_=relT_d[128 * c:128 * c + 128, :])
            Ecq = ph12w.tile([NB, 1024], F16, tag="Ecq")
            Eck = ph12w.tile([NB, 1024], F16, tag="Eck")
            nc.sync.dma_start(out=Ecq[:], in_=Ecq_d[:])
            nc.sync.dma_start(out=Eck[:], in_=Eck_d[:])

            # --- phase 1: LN1 overlapped with pos projection + M matrices ---
            with tc.tile_pool(name="ph1", bufs=3) as ph1, \
                 tc.tile_pool(name="ph1ps", bufs=4, space="PSUM") as ph1ps:
                for t in range(8):
                    xt = ph1.tile([128, H], F32, tag="x")
                    nc.sync.dma_start(out=xt[:], in_=hid_d[128 * t:128 * t + 128, :])
                    h16 = ph1.tile([128, H], F16, tag="h16")
                    layernorm(h16[:], xt[:], "scalar" if t % 2 else "vector")
                    nc.sync.dma_start(out=h_dram[128 * t:128 * t + 128, :], in_=h16[:])

                for fc in range(3):
                    ps = ph1ps.tile([128, 512], F32, tag="ps1")
                    for c in range(6):
                        nc.tensor.matmul(
                            ps[:NB, :], relT[:, c, :],
                            wqkT[:, c, 512 * fc:512 * fc + 512],
                            start=(c == 0), stop=(c == 5))
                    if fc == 0:
                        segs = [(0, 512, SCALE)]
                    elif fc == 1:
                        segs = [(0, 256, SCALE), (256, 512, 1.0)]
                    else:
                        segs = [(0, 512, 1.0)]
                    for (a, b_, sc) in segs:
                        if with_bias:
                            nc.vector.scalar_tensor_tensor(
                                out=posp[:NB, 512 * fc + a:512 * fc + b_],
                                in0=ps[:NB, a:b_], scalar=float(sc),
                                in1=bqkr[:NB, 512 * fc + a:512 * fc + b_],
                                op0=OP.mult, op1=OP.add)
                        else:
                            nc.vector.tensor_scalar_mul(
                                out=posp[:NB, 512 * fc + a:512 * fc + b_],
                                in0=ps[:NB, a:b_], scalar1=float(sc))
                for p in range(6):
                    for half in range(2):
                        hh = 2 * p + half
                        r0 = 64 * half
                        for ec in range(2):
                            ps = ph1ps.tile([128, 512], F32, tag="ps1")
                            nc.tensor.matmul(
                                ps[r0:r0 + 64, :],
                                posp[:NB, H + 64 * hh:H + 64 * hh + 64],
                                Ecq[:, 512 * ec:512 * ec + 512],
                                start=True, stop=True, tile_position=(0, r0))
                            nc.scalar.activation(
                                out=Mh[r0:r0 + 64, p, 512 * ec:512 * ec + 512],
                                in_=ps[r0:r0 + 64, :], func=AF.Copy)
                            ps2 = ph1ps.tile([128, 512], F32, tag="ps1")
                            nc.tensor.matmul(
                                ps2[r0:r0 + 64, :],
                                posp[:NB, 64 * hh:64 * hh + 64],
                                Eck[:, 512 * ec:512 * ec + 512],
                                start=True, stop=True, tile_position=(0, r0))
                            nc.vector.tensor_copy(
                                out=Mq[r0:r0 + 64, p, 512 * ec:512 * ec + 512],
                                in_=ps2[r0:r0 + 64, :])

            # transpose h by halves so QK(nh=0) can start after half A
            hT = ph12w.tile([128, 6, T], F16, tag="hT")
            for nh in range(2):
                for c in range(6):
                    nc.sync.dma_start_transpose(
                        out=hT[:, c, 512 * nh:512 * nh + 512],
                        in_=h_dram[512 * nh:512 * nh + 512, 128 * c:128 * c + 128])

            # --- phase 2: QK / VG projections ---
            with tc.tile_pool(name="ph2ps", bufs=4, space="PSUM") as ph2ps, \
                 tc.tile_pool(name="ph2", bufs=2) as ph2:
                for nh in range(2):
                    for f in range(12):
                        ps = ph2ps.tile([128, 512], F32, tag="ps2")
                        for c in range(6):
                            nc.tensor.matmul(
                                ps[:], wqkT[:, c, 128 * f:128 * f + 128],
                                hT[:, c, 512 * nh:512 * nh + 512],
                                start=(c == 0), stop=(c == 5))
                        if with_bias:
                            nc.scalar.activation(
                                out=qk16[:, f, 512 * nh:512 * nh + 512], in_=ps[:],
                                func=AF.Identity, bias=bqkc[:, f:f + 1],
                                scale=SCALE if f < 6 else 1.0)
                        else:
                            nc.scalar.activation(
                                out=qk16[:, f, 512 * nh:512 * nh + 512], in_=ps[:],
                                func=AF.Copy, bias=0.0,
                                scale=SCALE if f < 6 else 1.0)
                for t in range(8):
                    vg_t = ph2.tile([128, 2 * H], F16, tag="vg")
                    for fc in range(3):
                        ps = ph2ps.tile([128, 512], F32, tag="ps2")
                        for c in range(6):
                            nc.tensor.matmul(
                                ps[:], hT[:, c, 128 * t:128 * t + 128],
                                wvgT[:, c, 512 * fc:512 * fc + 512],
                                start=(c == 0), stop=(c == 5))
                        if with_bias:
                            nc.vector.scalar_tensor_tensor(
                                out=vg_t[:, 512 * fc:512 * fc + 512], in0=ps[:],
                                scalar=1.0,
                                in1=bvgr[:, 512 * fc:512 * fc + 512],
                                op0=OP.mult, op1=OP.add)
                        else:
                            nc.vector.tensor_copy(
                                out=vg_t[:, 512 * fc:512 * fc + 512], in_=ps[:])
                    nc.scalar.activation(out=g16[:, t, :], in_=vg_t[:, H:2 * H],
                                         func=AF.Gelu)
                    vav = va16[:, t, :].rearrange("p (h c) -> p h c", h=NH)
                    nc.vector.tensor_scalar_mul(
                        out=vav[:, :, 0:64],
                        in0=vg_t[:, 0:H].rearrange("p (h c) -> p h c", h=NH),
                        scalar1=vmask16[:, t:t + 1])
                    nc.vector.tensor_copy(
                        out=vav[:, :, 64],
                        in_=vmask16[:, t:t + 1].to_broadcast((128, NH)))

        # =================================================================
        # phase 3: attention per (b, head-pair)
        # =================================================================
        with tc.tile_pool(name="xps", bufs=1, space="PSUM") as xps, \
             tc.tile_pool(name="scps", bufs=1, space="PSUM") as scps, \
             tc.tile_pool(name="pvps", bufs=2, space="PSUM") as pvps, \
             tc.tile_pool(name="shp", bufs=3) as shp, \
             tc.tile_pool(name="cqp", bufs=9) as cqp, \
             tc.tile_pool(name="etp", bufs=12) as etp:

            def emit_pv_chunk(bprev, pprev, t, et_tiles):
                pv = pvps.tile([128, 132], F32, tag="pv",
                               padded_shape=[128, 512])
                for u in range(4):
                    for half in range(2):
                        hh = 2 * pprev + half
                        # NOTE: start=True wipes the whole PSUM bank, so only
                        # the very first matmul touching the bank may set it;
                        # the odd head's first matmul relies on cleared
                        # has_written bits to overwrite rather than accumulate.
                        nc.tensor.matmul(
                            pv[:, 66 * half:66 * half + 65],
                            et_tiles[u][:, half, 128 * t:128 * t + 128],
                            va16[:, 4 * bprev + u, 65 * hh:65 * hh + 65],
                            start=(u == 0 and half == 0), stop=(u == 3),
                            skip_group_check=True)
                rec2 = stats.tile([128, 2], F32, tag="rec2")
                nc.vector.reciprocal(out=rec2[:, 0:1], in_=pv[:, 64:65])
                nc.vector.reciprocal(out=rec2[:, 1:2], in_=pv[:, 130:131])
                pvv = pv[:].rearrange("p (h c) -> p h c", h=2)
                nc.vector.tensor_tensor(
                    out=ctx16[:, 4 * bprev + t,
                              128 * pprev:128 * pprev + 128].rearrange(
                                  "p (h c) -> p h c", h=2),
                    in0=pvv[:, :, 0:64],
                    in1=rec2[:].to_broadcast((128, 2, 64)),
                    op=OP.mult)

            def cq_round(st, t):
                b, p = st["b"], st["p"]
                tok0 = 512 * b
                ws = 384 - 128 * t
                xq_e = xps.tile([128, 640], F32, tag="xq_e",
                                padded_shape=[128, 1024])
                xq_o = xps.tile([128, 640], F32, tag="xq_o",
                                padded_shape=[128, 1024])
                for half, xq in ((0, xq_e), (1, xq_o)):
                    r0 = 64 * half
                    lq = qk16[r0:r0 + 64, p, tok0 + 128 * t:tok0 + 128 * t + 128]
                    nc.tensor.matmul(xq[:, 0:512], lq,
                                     Mh[r0:r0 + 64, p, ws:ws + 512],
                                     start=True, stop=True)
                    nc.tensor.matmul(xq[:, 512:640], lq,
                                     Mh[r0:r0 + 64, p, ws + 512:ws + 640],
                                     start=True, stop=True)
                wq = shp.tile([128, 2, 640], F16, tag="wq")
                nc.vector.tensor_copy(out=wq[:, 0, :], in_=xq_e[:])
                nc.scalar.activation(out=wq[:, 1, :], in_=xq_o[:], func=AF.Copy)
                cqt = cqp.tile([128, 2, 512], F16, tag="cq")
                nc.sync.dma_start(
                    out=cqt[:],
                    in_=bass.AP(tensor=wq.tensor, offset=wq.offset + 127,
                                ap=[[2 * 640 - 1, 128], [640, 2], [1, 512]]))
                st["cq"].append(cqt)

            def ck_round(st, t):
                b, p = st["b"], st["p"]
                tok0 = 512 * b
                ws = 384 - 128 * t
                xk_e = xps.tile([128, 640], F32, tag="xq_e",
                                padded_shape=[128, 1024])
                xk_o = xps.tile([128, 640], F32, tag="xq_o",
                                padded_shape=[128, 1024])
                for half, xk in ((0, xk_e), (1, xk_o)):
                    r0 = 64 * half
                    lk = qk16[r0:r0 + 64, 6 + p,
                              tok0 + 128 * t:tok0 + 128 * t + 128]
                    nc.tensor.matmul(xk[:, 0:512], lk,
                                     Mq[r0:r0 + 64, p, ws:ws + 512],
                                     start=True, stop=True)
                    nc.tensor.matmul(xk[:, 512:640], lk,
                                     Mq[r0:r0 + 64, p, ws + 512:ws + 640],
                                     start=True, stop=True)
                wk = shp.tile([128, 2, 640], F16, tag="wk")
                nc.scalar.activation(out=wk[:, 0, :], in_=xk_e[:], func=AF.Copy)
                nc.vector.tensor_copy(out=wk[:, 1, :], in_=xk_o[:])
                cksh = cqp.tile([128, 2, 512], F16, tag="cksh", bufs=9)
                nc.sync.dma_start(
                    out=cksh[:],
                    in_=bass.AP(tensor=wk.tensor, offset=wk.offset + 127,
                                ap=[[2 * 640 - 1, 128], [640, 2], [1, 512]]))
                st["ck"].append(cksh)

            def score_step(st, u):
                # scores^T for k-block u assembled in fp32 PSUM per head:
                # CC (start, wipes bank) + 4 transpose-by-matmul blocks of the
                # sheared cq (out = block.T @ I — a normal matmul, so it keeps
                # the HAM clock-gate warm, unlike transpose-mode) + identity-
                # add of the sheared ck; exp straight out of PSUM.
                b, p = st["b"], st["p"]
                tok0 = 512 * b
                et = etp.tile([128, 2, 512], F16, tag="et")
                for half in range(2):
                    r0 = 64 * half
                    sc = scps.tile([128, 512], F32,
                                   tag="sc_e" if half == 0 else "sc_o")
                    nc.tensor.matmul(
                        sc[:],
                        qk16[r0:r0 + 64, 6 + p, tok0 + 128 * u:tok0 + 128 * u + 128],
                        qk16[r0:r0 + 64, p, tok0:tok0 + 512],
                        start=True, stop=False, skip_group_check=True)
                    for t in range(4):
                        nc.tensor.matmul(
                            sc[:, 128 * t:128 * t + 128],
                            st["cq"][t][:, half, 128 * u:128 * u + 128],
                            ident16[:],
                            start=False, stop=False, skip_group_check=True)
                    nc.tensor.matmul(
                        sc[:], ident16[:], st["ck"][u][:, half, :],
                        start=False, stop=True, skip_group_check=True)
                    nc.scalar.activation(out=et[:, half, :], in_=sc[:],
                                         func=AF.Exp, bias=negc_t[:], scale=1.0)
                st["et"].append(et)

            pairs = [(b, p) for b in range(BL) for p in range(6)]
            NP = len(pairs)
            state = {}
            for slot in range(NP + 2):
                if slot < NP:
                    state[slot] = dict(b=pairs[slot][0], p=pairs[slot][1],
                                       cq=[], ck=[], et=[])
                for t in range(4):
                    if slot < NP:
                        cq_round(state[slot], t)
                    if 0 <= slot - 1 < NP:
                        score_step(state[slot - 1], t)
                    if slot < NP:
                        ck_round(state[slot], t)
                    if 0 <= slot - 2 < NP:
                        st2 = state[slot - 2]
                        emit_pv_chunk(st2["b"], st2["p"], t, st2["et"])
                if debug and slot == 1:
                    st0 = state[0]
                    for u in range(4):
                        nc.sync.dma_start(
                            out=dbg_et_d[:, 1024 * u:1024 * (u + 1)].rearrange(
                                "p (h c) -> p h c", h=2),
                            in_=st0["et"][u][:])
                        nc.sync.dma_start(
                            out=dbg_cq_d[:, 1024 * u:1024 * (u + 1)].rearrange(
                                "p (h c) -> p h c", h=2),
                            in_=st0["cq"][u][:])
                if slot - 2 >= 0:
                    state.pop(slot - 2, None)

            if debug:
                nc.sync.dma_start(out=dbg_qk_d[:], in_=qk16[:])
                nc.sync.dma_start(out=dbg_va_d[:], in_=va16[:])
                nc.sync.dma_start(out=dbg_g_d[:], in_=g16[:])
                nc.sync.dma_start(out=dbg_ctx_d[:], in_=ctx16[:])

        # =================================================================
        # phase 4: gate, LN2, out projection
        # =================================================================
        with tc.tile_pool(name="ph4ps", bufs=3, space="PSUM") as ph4ps, \
             tc.tile_pool(name="ph4", bufs=2) as ph4, \
             tc.tile_pool(name="ph4w", bufs=1) as ph4w:
            ln2T = ph4w.tile([128, 6, T], F16, tag="ln2T")
            for half in range(2):
                for t in range(4 * half, 4 * half + 4):
                    cg = ph4.tile([128, H], F16, tag="cg")
                    nc.vector.tensor_mul(cg[:], ctx16[:, t, :], g16[:, t, :])
                    ln2 = ph4.tile([128, H], F16, tag="ln2")
                    layernorm(ln2[:], cg[:], "scalar" if t % 2 else "vector")
                    nc.sync.dma_start(out=ln2_dram[128 * t:128 * t + 128, :],
                                      in_=ln2[:])
                for c in range(6):
                    nc.sync.dma_start_transpose(
                        out=ln2T[:, c, 512 * half:512 * half + 512],
                        in_=ln2_dram[512 * half:512 * half + 512,
                                     128 * c:128 * c + 128])
                for t in range(4 * half, 4 * half + 4):
                    ot = ph4.tile([128, H], F32, tag="ot")
                    for fc, (f0, fw) in enumerate([(0, 512), (512, 256)]):
                        ps = ph4ps.tile([128, 512], F32, tag="ops")
                        for c in range(6):
                            nc.tensor.matmul(
                                ps[:, :fw], ln2T[:, c, 128 * t:128 * t + 128],
                                woutT[:, c, f0:f0 + fw],
                                start=(c == 0), stop=(c == 5))
                        if with_bias:
                            nc.vector.scalar_tensor_tensor(
                                out=ot[:, f0:f0 + fw], in0=ps[:, :fw], scalar=1.0,
                                in1=boutr[:, f0:f0 + fw], op0=OP.mult, op1=OP.add)
                        elif fc == 0:
                            nc.vector.tensor_copy(out=ot[:, f0:f0 + fw],
                                                  in_=ps[:, :fw])
                        else:
                            nc.scalar.activation(out=ot[:, f0:f0 + fw],
                                                 in_=ps[:, :fw], func=AF.Copy)
                    nc.sync.dma_start(out=out_d[128 * t:128 * t + 128, :],
                                      in_=ot[:])

    return nc


# ---------------------------------------------------------------------------
# host side
# ---------------------------------------------------------------------------
def _host_prep(position_indices, attention_mask):
    pi = np.asarray(position_indices)
    gvec = np.empty(1023, np.int64)
    gvec[511:] = pi[:, 0]
    gvec[:512] = pi[0, ::-1]
    d = np.arange(S)[:, None] - np.arange(S)[None, :]
    assert np.array_equal(gvec[d + 511], pi), "position_indices not Toeplitz"
    e = np.arange(1023)
    E_cq = (np.arange(NB)[:, None] == gvec[1022 - e][None, :]).astype(np.float16)
    E_ck = (np.arange(NB)[:, None] == gvec[e][None, :]).astype(np.float16)
    E_cq = np.concatenate([E_cq, np.zeros((NB, 1), np.float16)], 1)
    E_ck = np.concatenate([E_ck, np.zeros((NB, 1), np.float16)], 1)
    am = np.asarray(attention_mask).reshape(B, S)
    vmask = (~am).astype(np.float32)
    return E_cq, E_ck, vmask


def kernel(hidden_states, relative_embedding, w_qk, b_qk, w_vg, b_vg,
           w_out, b_out, attention_mask, position_indices):
    from concourse.bass_utils import run_bass_kernel_spmd

    hidden_states = np.asarray(hidden_states, dtype=np.float32)
    relative_embedding = np.asarray(relative_embedding, dtype=np.float32)
    w_qk = np.asarray(w_qk, dtype=np.float32)
    w_vg = np.asarray(w_vg, dtype=np.float32)
    w_out = np.asarray(w_out, dtype=np.float32)
    b_qk = np.asarray(b_qk, dtype=np.float32)
    b_vg = np.asarray(b_vg, dtype=np.float32)
    b_out = np.asarray(b_out, dtype=np.float32)

    with_bias = bool(np.any(b_qk) or np.any(b_vg) or np.any(b_out))
    E_cq, E_ck, vmask = _host_prep(position_indices, attention_mask)

    nc = build_module(with_bias)
    common = dict(
        wqkT=np.ascontiguousarray(w_qk.T).astype(np.float16),
        wvgT=np.ascontiguousarray(w_vg.T).astype(np.float16),
        woutT=np.ascontiguousarray(w_out.T).astype(np.float16),
        relT=np.ascontiguousarray(relative_embedding.T).astype(np.float16),
        Ecq=E_cq, Eck=E_ck)
    if with_bias:
        sc_col = np.where(np.arange(12) < 6, SCALE, 1.0).astype(np.float32)
        common["bqkc"] = np.ascontiguousarray(
            b_qk.reshape(12, 128).T * sc_col[None, :])
        sc_row = np.concatenate([np.full(H, SCALE), np.ones(H)]).astype(np.float32)
        common["bqkr"] = (b_qk * sc_row)[None, :].astype(np.float32)
        common["bvgr"] = b_vg[None, :].astype(np.float32)
        common["boutr"] = b_out[None, :].astype(np.float32)

    in_maps = []
    for core in range(NCORES):
        bsel = [BL * core + i for i in range(BL)]
        hid = np.ascontiguousarray(
            hidden_states[:, bsel, :].transpose(1, 0, 2).reshape(T, H))
        vm = np.ascontiguousarray(vmask[bsel].reshape(T, 1))
        in_maps.append(dict(common, hid=hid, vmask=vm))

    res = run_bass_kernel_spmd(nc, in_maps, list(range(NCORES)))
    out = np.empty((S, B, H), np.float32)
    for core in range(NCORES):
        o = res.results[core]["out"].reshape(BL, S, H)
        for i in range(BL):
            out[:, BL * core + i, :] = o[i]
    return out
